# revision 1
# baseline (speedup 1.0000x reference)
"""Trainium2 Bass kernel for nn_CascadedAttention_76836964925817.

Math: the reference module's attention machinery is dead code — softmax over a
size-1 axis is identically 1, so `context = x[0].sum(axis=0)` is a constant
and the layer reduces to the 28-dim nonlinear recurrence

    y[t] = sigmoid(Wo @ y[t-1] + Uo @ x[t-1] + c),   c = Co @ sum_t x[t],
    y[-1] = 0, x[-1] := 0.

Strategy:
  * Precompute B[t] = Uo @ x[t-1] (a (2048, 28) matrix) and c on device.
    This phase is sharded over T across the 8 cores (each core handles 256
    timesteps of x, pre-transposed/interleaved on the host so the contraction
    dim D lands on SBUF partitions with one fully-contiguous DMA), then an
    AllGather shares the per-core (28 x 256) results + partial c sums.
  * Solve the recurrence by fixed-point (Jacobi) iteration:
        Y <- sigmoid(shift(Y) @ Wo.T + B + c)
    The map is a strong contraction (|sigmoid'| <= 1/4, ||Wo|| ~ 0.53;
    empirically the error floor is reached after 2-3 sweeps).
  * Iteration layout: t is split into 4 column groups of 512 stacked on
    partition blocks 28g..28g+27 (112 active partitions).  Each sweep is one
    three-matmul accumulation chain in fp32r (1 cycle/column on the PE):
        MM1: psum  = I112 @ bg                         (B term; bg pre-shifted)
        MM2: psum += blockdiag(Wo.T) @ YA[:, 0:512]    (shifted-y storage)
        MM3: psum += shiftblk(Wo.T) @ YA[:, 512:514]   (group boundary;
             col 513 is a permanent zero so the 2-col dst stays fp32r-legal)
    then one 112-lane sigmoid ACT with per-partition bias c writes
    YA[:, 1:513].  fp32r dst rules (start partition 0, even column count,
    8B alignment) hold by construction; masks are zero-padded host weights.

The kernel is self-contained: shapes/sharding are hardcoded.
"""

import numpy as np

import concourse.bass as bass
import concourse.mybir as mybir
import concourse.tile as tile
from concourse import bacc
from concourse import bass_utils

F32 = mybir.dt.float32
F32R = mybir.dt.float32r
BF16 = mybir.dt.bfloat16
AF = mybir.ActivationFunctionType

T, D, V = 2048, 1024, 28
N_CORES = 8
TC = T // N_CORES          # 256 timesteps per core in the B-precompute phase
G = 4                      # column groups in the iteration phase
S = T // G                 # 512 columns per group
P4 = G * V                 # 112 active partitions in the iteration phase
DCH = D // 128             # 8 contraction chunks
N_ITERS = 3                # fixed-point refinement sweeps (after the init sweep)
W2 = 64                    # padded [Uo;Co] output rows: Uo 0:28, Co 32:60
TH = TC + 2                # per-core timestep window incl. 2-col halo (even)

USE_F32R = True
USE_CC = True              # AllGather on; off = single-core-data debug mode


def build_body(nc, xt, w2t, wmm, eye, yg, n_iters=N_ITERS, tc=None,
               reps=1):
    """Emit the program. xt:(128, 8*256) x chunk, d-major interleaved;
    w2t:(1024,64) zero-padded [Uo;Co].T; wmm:(112, 3, 112) block weights
    ([.,0,.]=I112, [.,1,.]=blockdiag(Wo.T), [.,2,.]=boundary-shift(Wo.T));
    yg:(112,512) grouped output."""
    t = tc
    from contextlib import ExitStack
    ctx = ExitStack()
    sbp = ctx.enter_context(t.tile_pool(name="sb", bufs=1))
    pp = ctx.enter_context(t.tile_pool(name="pp", bufs=1, space="PSUM"))
    dp = ctx.enter_context(t.tile_pool(name="dp", bufs=2, space="DRAM"))

    MDT = F32R if USE_F32R else F32

    def st(shape, name, dt=F32):
        return sbp.tile(shape, dt, name=name, tag=name)

    xt_sb = st([128, 2, DCH, TH], "xt_sb", BF16)
    w2t_sb = st([128, 2, DCH, W2], "w2t_sb", BF16)
    wmm_sb = st([P4, 2, P4], "wmm_sb", MDT)
    eye_sb = st([P4, P4], "eye_sb", BF16)
    usb = st([W2, 2, TH], "usb", BF16)
    cpart = st([W2, 1], "cpart")
    cprt_bf = st([W2, 2], "cprt_bf", BF16)
    csb = st([P4, 2 * N_CORES], "csb", BF16)
    cbias = st([P4, 1], "cbias")
    bg = st([P4, 2, S], "bg", BF16)
    ya = st([P4, S + 2], "ya", MDT)
    yfin = st([P4, S], "yfin")
    dummy = st([1, 1], "dummy")

    upsum = pp.tile([W2, TH], F32, name="upsum", tag="upsum")
    psa = pp.tile([P4, S], F32, name="psa", tag="psa")
    psb = pp.tile([P4, S], F32, name="psb", tag="psb")

    # Early dummy sigmoid so the ACT table load happens off the critical path.
    nc.vector.memset(dummy[:, :], 0.0)
    nc.scalar.activation(out=dummy[:, :], in_=dummy[:, :], func=AF.Sigmoid)

    # one-time constants
    nc.sync.dma_start(wmm_sb[:, :, :], wmm)
    nc.sync.dma_start(eye_sb[:, :], eye)
    nc.sync.dma_start(w2t_sb[:, :, :, :],
                      w2t.rearrange("p (h c v) -> p h c v", h=2, c=DCH))
    nc.vector.memset(bg[:, :, :].bitcast(mybir.dt.uint16), 0)
    nc.vector.memset(ya[:, :].bitcast(F32), 0.0)

    prev_last = None
    for _rep in range(reps):
        prev_last = emit_rep(nc, t, dp, xt, yg, n_iters,
                             xt_sb, w2t_sb, wmm_sb, eye_sb, usb,
                             cpart, cprt_bf, csb, cbias, bg, ya, yfin,
                             upsum, psa, psb, prev_last)
    ctx.close()


def emit_rep(nc, t, dp, xt, yg, n_iters,
             xt_sb, w2t_sb, wmm_sb, eye_sb, usb, cpart, cprt_bf, csb,
             cbias, bg, ya, yfin, upsum, psa, psb, prev_last=None):
    from concourse.tile_rust import add_dep_helper
    MDT = F32R if USE_F32R else F32
    pay = dp.tile([V, 2 * TH + 2], BF16, name="pay", tag="pay")
    agout = dp.tile([V * N_CORES, 2 * TH + 2], BF16, name="agout",
                    tag="agout", addr_space="Shared")

    # ---------------- load x chunk (one fully-contiguous 1MB DMA) ----------
    xdma = nc.sync.dma_start(xt_sb[:, :, :, :],
                             xt.rearrange("p (h c t) -> p h c t", h=2, c=DCH))
    if prev_last is not None:
        add_dep_helper(xdma.ins, prev_last.ins,
                       reason="serialize reps for latency measurement")

    # -------- U = [Uo;Co] @ x_chunk.T  -> (64, 258), bf16 hi/lo split ------
    terms = [(0, 0), (0, 1), (1, 0)]   # (w half, x half); lo*lo dropped
    nmm = DCH * len(terms)
    i = 0
    for c in range(DCH):
        for hw, hx in terms:
            i += 1
            nc.tensor.matmul(
                upsum[:, :],
                lhsT=w2t_sb[:, hw, c, :],
                rhs=xt_sb[:, hx, c, :],
                start=(i == 1),
                stop=(i == nmm),
            )
    nc.vector.tensor_copy(usb[:, 0, :], upsum[:, :])
    nc.vector.tensor_tensor(usb[:, 1, :], upsum[:, :], usb[:, 0, :],
                            mybir.AluOpType.subtract)
    # partial c: row-sums of the Co part (own timesteps only, not the halo)
    nc.vector.tensor_reduce(
        out=cpart[32:32 + V, :], in_=upsum[32:32 + V, 2:TH],
        axis=mybir.AxisListType.X, op=mybir.AluOpType.add,
    )
    nc.vector.tensor_copy(cprt_bf[32:32 + V, 0:1], cpart[32:32 + V, :])
    nc.vector.tensor_tensor(cprt_bf[32:32 + V, 1:2], cpart[32:32 + V, :],
                            cprt_bf[32:32 + V, 0:1],
                            mybir.AluOpType.subtract)

    # ---------------- AllGather U chunks + partial c ----------------
    nc.sync.dma_start(pay[0:V, 0:2 * TH], usb[0:V, :, :])
    nc.sync.dma_start(pay[0:V, 2 * TH:2 * TH + 2], cprt_bf[32:32 + V, :])
    if USE_CC:
        nc.gpsimd.collective_compute(
            "AllGather",
            mybir.AluOpType.bypass,
            replica_groups=[list(range(N_CORES))],
            ins=[pay.opt()],
            outs=[agout.opt()],
        )
    else:
        nc.sync.dma_start(agout[0:V, :], pay[:, :])

    # ---------------- assemble grouped B and c ----------------
    # bg[28g+v, tau] = U[512g + tau - 1, v].  Core r's payload col j holds
    # U[256r - 2 + j] (2-col halo, core 0's halo is zero), so group g is
    # [core 2g cols 1:258 | core 2g+1 cols 2:257] with no boundary fixups.
    # Two full-112-partition DMAs: flat SBUF dst, (4,28,cols) DRAM src.
    # c = sum over cores of partial c; the (112 x 16) tile holds the hi/lo
    # partials replicated per partition group so one reduce yields the bias
    csrc = agout.opt().rearrange("(r p) f -> p r f", p=V)[0:V, :,
                                                          2 * TH:2 * TH + 2]
    for g in range(G):
        nc.sync.dma_start(csb[V * g:V * g + V, :], csrc)
    nc.vector.tensor_reduce(out=cbias[:, :], in_=csb[:, :],
                            axis=mybir.AxisListType.X, op=mybir.AluOpType.add)

    agv = agout.opt().rearrange("(r p) f -> r p f", p=V)
    for h in range(2):
        o = h * TH
        nc.sync.dma_start(bg[0:P4, h, 0:TC + 1],
                          agv[0:2 * G:2, :, o + 1:o + TH])
        nc.sync.dma_start(bg[0:P4, h, TC + 1:S],
                          agv[1:2 * G:2, :, o + 2:o + TC + 1])

    # ---------------- fixed-point iterations ----------------
    # YA[28g+v, j] stores y[512g + j - 1] for j in 1..512; col 0 and col 513
    # are permanent zeros (memset once).  psum col tau = z[512g + tau] before
    # the bias; ACT writes sigmoid(psum + c) into YA[:, 1:513].
    for k in range(n_iters + 1):
        ps = psa if k % 2 == 0 else psb
        for h in range(2):
            nc.tensor.matmul(
                ps[:, :],
                lhsT=eye_sb[:, :],
                rhs=bg[:, h, :],
                start=(h == 0), stop=(k == 0 and h == 1),
            )
        if k > 0:
            nc.tensor.matmul(
                ps[:, :],
                lhsT=wmm_sb[:, 0, :],
                rhs=ya[:, 0:S],
                start=False, stop=False,
            )
            nc.tensor.matmul(
                ps[:, 0:2],
                lhsT=wmm_sb[:, 1, :],
                rhs=ya[:, S:S + 2],
                start=False, stop=True,
            )
        if k < n_iters:
            nc.scalar.activation(out=ya[:, 1:S + 1], in_=ps[:, :],
                                 func=AF.Sigmoid, bias=cbias[:, 0:1],
                                 scale=1.0)
        else:
            nc.scalar.activation(out=yfin[:, :], in_=ps[:, :],
                                 func=AF.Sigmoid, bias=cbias[:, 0:1],
                                 scale=1.0)

    # ---------------- write grouped output ----------------
    return nc.sync.dma_start(yg, yfin[:, :])


_CACHED_NC = {}


def _get_nc(reps=1):
    if reps not in _CACHED_NC:
        nc = bacc.Bacc("TRN2", target_bir_lowering=False, debug=False,
                       num_devices=N_CORES)
        MDT = F32R if USE_F32R else F32
        xt = nc.dram_tensor("xt", [128, 2 * DCH * TH], BF16,
                            kind="ExternalInput")
        w2t = nc.dram_tensor("w2t", [128, 2 * DCH * W2], BF16,
                             kind="ExternalInput")
        wmm = nc.dram_tensor("wmm", [P4, 2, P4], MDT, kind="ExternalInput")
        eye = nc.dram_tensor("eye", [P4, P4], BF16, kind="ExternalInput")
        yg = nc.dram_tensor("yg", [P4, S], F32, kind="ExternalOutput")
        with tile.TileContext(nc) as t:
            build_body(nc, xt.ap(), w2t.ap(), wmm.ap(), eye.ap(), yg.ap(),
                       tc=t, reps=reps)
        nc.compile()
        _CACHED_NC[reps] = nc
    return _CACHED_NC[reps]


def _hilo(a):
    """Split fp32 array into (hi, lo) bf16 parts: a ~ hi + lo."""
    import ml_dtypes
    hi = a.astype(ml_dtypes.bfloat16)
    lo = (a - hi.astype(np.float32)).astype(ml_dtypes.bfloat16)
    return hi, lo


def make_in_maps(x, Uo, Co, Wo):
    import ml_dtypes
    xb = np.ascontiguousarray(np.asarray(x, np.float32)[0])        # (T, D)
    w2 = np.zeros((W2, D), np.float32)
    w2[0:V] = np.asarray(Uo, np.float32)
    w2[32:32 + V] = np.asarray(Co, np.float32)
    w2tf = np.ascontiguousarray(
        w2.T.reshape(DCH, 128, W2).transpose(1, 0, 2))             # (128,8,64)
    w2h, w2l = _hilo(w2tf)
    w2t = np.ascontiguousarray(
        np.stack([w2h, w2l], axis=1).reshape(128, 2 * DCH * W2))
    wot1 = np.ascontiguousarray(np.asarray(Wo, np.float32).T)      # (V, V)
    wmm = np.zeros((P4, 2, P4), np.float32)
    for g in range(G):
        wmm[V * g:V * g + V, 0, V * g:V * g + V] = wot1
        if g > 0:
            wmm[V * (g - 1):V * (g - 1) + V, 1, V * g:V * g + V] = wot1
    eye = np.eye(P4, dtype=ml_dtypes.bfloat16)
    in_maps = []
    for r in range(N_CORES):
        xh = np.zeros((TH, D), np.float32)                         # (258, D)
        lo = r * TC - 2
        xh[max(0, -lo):, :] = xb[max(0, lo):(r + 1) * TC, :]
        xc = np.ascontiguousarray(
            xh.T.reshape(DCH, 128, TH).transpose(1, 0, 2))         # (128,8,258)
        xhi, xlo = _hilo(xc)
        xi = np.ascontiguousarray(
            np.stack([xhi, xlo], axis=1).reshape(128, 2 * DCH * TH))
        in_maps.append({"xt": xi, "w2t": w2t, "wmm": wmm, "eye": eye})
    return in_maps


def unshard_output(yg):
    y = np.empty((T, V), np.float32)
    for g in range(G):
        y[g * S:(g + 1) * S, :] = yg[V * g:V * g + V, :].T
    return y[None]


def run(inputs, trace=False, reps=1, **kw):
    nc = _get_nc(reps)
    in_maps = make_in_maps(inputs["x"], inputs["Uo"], inputs["Co"],
                           inputs["Wo"])
    res = bass_utils.run_bass_kernel_spmd(
        nc, in_maps, core_ids=list(range(N_CORES)), trace=trace, **kw)
    return unshard_output(res.results[0]["yg"]), res


def kernel(**inputs):
    out, _ = run(inputs)
    return out



# revision 4
# speedup vs baseline: 1.8596x; 1.8596x over previous
"""Trainium2 Bass kernel for nn_CascadedAttention_76836964925817.

Math: the reference module's attention machinery is dead code — softmax over a
size-1 axis is identically 1, so `context = x[0].sum(axis=0)` is a constant
and the layer reduces to the 28-dim nonlinear recurrence

    y[t] = sigmoid(Wo @ y[t-1] + Uo @ x[t-1] + c),   c = Co @ sum_t x[t],
    y[-1] = 0, x[-1] := 0.

The map y -> sigmoid(Wo y + b) is a strong contraction (measured Jacobian
2-norm <= 0.055), so each core can solve its own 256-timestep slice from a
cold start with a W=4 column warmup — no cross-core state is needed.

Collective-free design (the previous kernel's AllGather dominated its
runtime through inter-core rendezvous): every core receives the FULL x
(8.4 MB fp32, column-permuted on the host so one SPMD program works for all
cores) and computes the global sum itself:

  * per-core input xall (128, 8, 2054): d-major chunks; cols [0,260) are the
    core's local window x[t0-4 .. t0+254] (fed to the U matmuls), cols
    [260,2054) are all remaining timesteps in arbitrary order.  The global
    sum_t x[t] is 8 free-axis VectorE reduces over cols [3,2054), pipelined
    behind the 8 per-chunk DMAs.
  * U window: 8 fp32 matmuls (Uo.T chunks vs window cols) accumulated in
    PSUM; one extra identity matmul adds E (E = -500 on warmup cols for
    core 0 only, making its pre-t=0 state decay to the true zero init).
  * c: 8 tiny matmuls Co.T chunks vs the reduced sums.
  * recurrence: B' = U + c assembled with one per-partition-bias ACT; a
    sigmoid warm-init ACT; then S=3 Jacobi sweeps, each one matmul with the
    constant stationary [Wo.T; I28] over the stacked [Y; B'] window plus one
    sigmoid ACT.  Output is Y cols [4,260) = y[t0 .. t0+255].
"""

import numpy as np

import concourse.bass as bass
import concourse.mybir as mybir
import concourse.tile as tile
from concourse import bacc
from concourse import bass_utils

F32 = mybir.dt.float32
AF = mybir.ActivationFunctionType

T, D, V = 2048, 1024, 28
N_CORES = 8
TC = T // N_CORES        # 256 output timesteps per core
W = 4                    # warmup columns
NW = TC + W              # 260 window columns (U matmul width, even)
XCW = 2054               # per-chunk input cols: 260 window + 1794 complement
DCH = D // 128           # 8 contraction chunks
S_SWEEPS = 3             # Jacobi sweeps after the sigmoid warm-init
E_NEG = -500.0           # warmup bias (must be < -(max|c| + margin) ~ -170)


def build_body(nc, xall, uot, cot, wois, ident, esrc, yg, tc=None):
    t = tc
    from contextlib import ExitStack
    ctx = ExitStack()
    sbp = ctx.enter_context(t.tile_pool(name="sb", bufs=1))
    pp = ctx.enter_context(t.tile_pool(name="pp", bufs=1, space="PSUM"))

    def st(shape, name):
        return sbp.tile(shape, F32, name=name, tag=name)

    xall_sb = st([128, DCH, XCW], "xall_sb")
    uot_sb = st([128, DCH, V], "uot_sb")
    cot_sb = st([128, DCH, V], "cot_sb")
    wois_sb = st([64, V], "wois_sb")
    ident_sb = st([V, V], "ident_sb")
    e_sb = st([V, W], "e_sb")
    sred = st([128, DCH], "sred")
    cbias = st([V, 1], "cbias")
    m_sb = st([64, NW + 2], "m_sb")
    dummy = st([1, 1], "dummy")

    psU = pp.tile([V, NW], F32, name="psU", tag="psU")
    psC = pp.tile([V, 1], F32, name="psC", tag="psC")
    psZ = pp.tile([V, NW], F32, name="psZ", tag="psZ")

    # Early dummy sigmoid so the ACT table load happens off the critical path.
    nc.vector.memset(dummy[:, :], 0.0)
    nc.scalar.activation(out=dummy[:, :], in_=dummy[:, :], func=AF.Sigmoid)
    # Y region must start as zeros (cold-start warmup state).
    nc.vector.memset(m_sb[:, :], 0.0)

    # ---------------- DMAs: small constants, then x chunk streams ----------
    nc.sync.dma_start(uot_sb[:, :, :], uot.rearrange("p (c v) -> p c v", c=DCH))
    nc.sync.dma_start(cot_sb[:, :, :], cot.rearrange("p (c v) -> p c v", c=DCH))
    nc.sync.dma_start(wois_sb[:, :], wois)
    nc.sync.dma_start(ident_sb[:, :], ident)
    nc.sync.dma_start(e_sb[:, :], esrc)
    xv = xall.rearrange("p (c j) -> p c j", c=DCH)
    for c in range(DCH):
        nc.sync.dma_start(xall_sb[:, c, :], xv[:, c, :])

    # ---------------- U = Uo @ window, + E on the warmup cols --------------
    for c in range(DCH):
        nc.tensor.matmul(
            psU[:, :],
            lhsT=uot_sb[:, c, :],
            rhs=xall_sb[:, c, 0:NW],
            start=(c == 0),
            stop=False,
        )
    nc.tensor.matmul(
        psU[:, 0:W],
        lhsT=ident_sb[:, :],
        rhs=e_sb[:, :],
        start=False,
        stop=True,
    )

    # ---------------- global sum + c --------------------------------------
    # Each chunk's reduce covers window cols [3,260) (every t in
    # [t0-1, t0+254] exactly once; cols 0-2 are U-halo duplicates) plus the
    # complement cols — together every timestep exactly once.
    for c in range(DCH):
        nc.vector.tensor_reduce(
            out=sred[:, c:c + 1], in_=xall_sb[:, c, 3:XCW],
            axis=mybir.AxisListType.X, op=mybir.AluOpType.add,
        )
    for c in range(DCH):
        nc.tensor.matmul(
            psC[:, :],
            lhsT=cot_sb[:, c, :],
            rhs=sred[:, c:c + 1],
            start=(c == 0),
            stop=(c == DCH - 1),
        )
    nc.vector.tensor_copy(cbias[:, :], psC[:, :])

    # ---------------- assemble B' rows and sigmoid warm-init ---------------
    nc.scalar.activation(out=m_sb[32:32 + V, 0:NW], in_=psU[:, :],
                         func=AF.Identity, bias=cbias[:, 0:1], scale=1.0)
    nc.scalar.activation(out=m_sb[0:V, 1:NW + 1], in_=psU[:, :],
                         func=AF.Sigmoid, bias=cbias[:, 0:1], scale=1.0)

    # ---------------- Jacobi sweeps ---------------------------------------
    for _ in range(S_SWEEPS):
        nc.tensor.matmul(
            psZ[:, :],
            lhsT=wois_sb[:, :],
            rhs=m_sb[0:64, 0:NW],
            start=True,
            stop=True,
        )
        nc.scalar.activation(out=m_sb[0:V, 1:NW + 1], in_=psZ[:, :],
                             func=AF.Sigmoid)

    # ---------------- write output ----------------------------------------
    nc.sync.dma_start(yg, m_sb[0:V, W:NW])
    ctx.close()


_CACHED_NC = {}


def _get_nc():
    if "nc" not in _CACHED_NC:
        nc = bacc.Bacc("TRN2", target_bir_lowering=False, debug=False,
                       num_devices=N_CORES)
        xall = nc.dram_tensor("xall", [128, DCH * XCW], F32,
                              kind="ExternalInput")
        uot = nc.dram_tensor("uot", [128, DCH * V], F32, kind="ExternalInput")
        cot = nc.dram_tensor("cot", [128, DCH * V], F32, kind="ExternalInput")
        wois = nc.dram_tensor("wois", [64, V], F32, kind="ExternalInput")
        ident = nc.dram_tensor("ident", [V, V], F32, kind="ExternalInput")
        esrc = nc.dram_tensor("esrc", [V, W], F32, kind="ExternalInput")
        yg = nc.dram_tensor("yg", [V, TC], F32, kind="ExternalOutput")
        with tile.TileContext(nc) as t:
            build_body(nc, xall.ap(), uot.ap(), cot.ap(), wois.ap(),
                       ident.ap(), esrc.ap(), yg.ap(), tc=t)
        nc.compile()
        _CACHED_NC["nc"] = nc
    return _CACHED_NC["nc"]


def _to_dev_layout(buf):
    """(cols, D) -> (128, DCH*cols): dev[p, c*cols+j] = buf[j, 128c+p]."""
    cols = buf.shape[0]
    return np.ascontiguousarray(
        buf.T.reshape(DCH, 128, cols).transpose(1, 0, 2).reshape(128, -1))


def make_in_maps(x, Uo, Co, Wo):
    xb = np.ascontiguousarray(np.asarray(x, np.float32)[0])        # (T, D)
    Uo = np.asarray(Uo, np.float32)
    Co = np.asarray(Co, np.float32)
    Wo = np.asarray(Wo, np.float32)

    uot = _to_dev_layout(Uo)                                       # (128, 8*28)
    cot = _to_dev_layout(Co)
    wois = np.zeros((64, V), np.float32)
    wois[0:V] = Wo.T
    wois[32:32 + V] = np.eye(V, dtype=np.float32)
    ident = np.eye(V, dtype=np.float32)

    in_maps = []
    for r in range(N_CORES):
        t0 = r * TC
        buf = np.zeros((XCW, D), np.float32)
        # window cols w=0..258 <-> x[t0-4+w]; col 259 stays zero
        lo = t0 - W
        src_lo = max(0, lo)
        buf[src_lo - lo:NW - 1] = xb[src_lo:t0 + TC - 1]
        # complement: every t outside [t0-1, t0+254]
        comp = np.concatenate([np.arange(0, max(0, t0 - 1)),
                               np.arange(t0 + TC - 1, T)])
        buf[NW:NW + len(comp)] = xb[comp]
        esrc = np.zeros((V, W), np.float32)
        if r == 0:
            esrc[:, 0:W - 1] = E_NEG
        in_maps.append({
            "xall": _to_dev_layout(buf),
            "uot": uot, "cot": cot, "wois": wois, "ident": ident,
            "esrc": esrc,
        })
    return in_maps


def unshard_output(results):
    y = np.empty((T, V), np.float32)
    for r in range(N_CORES):
        y[r * TC:(r + 1) * TC, :] = results[r]["yg"].T
    return y[None]


def run(inputs, trace=False, **kw):
    nc = _get_nc()
    in_maps = make_in_maps(inputs["x"], inputs["Uo"], inputs["Co"],
                           inputs["Wo"])
    res = bass_utils.run_bass_kernel_spmd(
        nc, in_maps, core_ids=list(range(N_CORES)), trace=trace, **kw)
    return unshard_output(res.results), res


def kernel(**inputs):
    out, _ = run(inputs)
    return out


# revision 8
# speedup vs baseline: 2.1588x; 1.1609x over previous
"""Trainium2 Bass kernel for nn_CascadedAttention_76836964925817.

Math: the reference module's attention machinery is dead code — softmax over a
size-1 axis is identically 1, so `context = x[0].sum(axis=0)` is a constant
and the layer reduces to the 28-dim nonlinear recurrence

    y[t] = sigmoid(Wo @ y[t-1] + Uo @ x[t-1] + c),   c = Co @ sum_t x[t],
    y[-1] = 0, x[-1] := 0.

The map y -> sigmoid(Wo y + b) is a strong contraction (measured Jacobian
2-norm <= 0.055), so each core solves its own 256-timestep slice from a cold
start with a W=4 column warmup — no cross-core state is needed.

Collective-free design (a collective's rendezvous wait absorbs inter-core
launch skew into the first core's measured exec time): every core receives
the FULL x (8.4 MB fp32, column-permuted on the host so one SPMD program
works for all cores) and computes the global sum itself:

  * per-core input xall (128, 8, 2054): d-major chunks; cols [0,260) are the
    core's local window x[t0-4 .. t0+254] (fed to the U matmuls), cols
    [260,2054) are all remaining timesteps in arbitrary order.
  * The global sum_t x[t] is one free-axis sum per chunk over cols [3,2054),
    split across VectorE (tensor_reduce), ScalarE (activation accum_out) and
    GpSimd (tensor_scalar accum_out) so every chunk's sum hides behind the
    per-chunk DMA stream; the last-arriving chunk goes to a fast engine.
  * U window: 8 f32r matmuls (Uo.T chunks vs window cols) accumulated in
    PSUM; one extra identity matmul adds E (E = -500 on warmup cols for
    core 0 only, making its pre-t=0 state decay to the true zero init).
  * c: 8 tiny f32 matmuls Co.T chunks vs the per-chunk sums.
  * recurrence: B' = U + c assembled with one per-partition-bias ACT; a
    sigmoid warm-init ACT; then S=2 Jacobi sweeps, each one f32r matmul with
    the constant stationary [Wo.T;0;I28;0] over the stacked [Y; B'] window
    plus one sigmoid ACT.  Output is Y cols [4,260) = y[t0 .. t0+255].

All constants ride in a single packed (128, 508) tensor -> one DMA.
"""

import numpy as np

import concourse.bass as bass
import concourse.mybir as mybir
import concourse.tile as tile
from concourse import bacc
from concourse import bass_utils

F32 = mybir.dt.float32
F32R = mybir.dt.float32r
AF = mybir.ActivationFunctionType
ALU = mybir.AluOpType

T, D, V = 2048, 1024, 28
N_CORES = 8
TC = T // N_CORES        # 256 output timesteps per core
W = 4                    # warmup columns
NW = TC + W              # 260 window columns (U matmul width, even)
XCW = 2054               # per-chunk input cols: 260 window + 1794 complement
DCH = D // 128           # 8 contraction chunks
S_SWEEPS = 2             # Jacobi sweeps after the sigmoid warm-init
E_NEG = -500.0           # warmup bias (must be < -(max|c| + margin) ~ -170)
USE_F32R = True

# packed consts layout (128, 508)
C_UOT = 0                # [0, 224): Uo.T chunks
C_COT = DCH * V          # [224, 448): Co.T chunks
C_WOIS = 2 * DCH * V     # [448, 476): [Wo.T; 0; I28; 0] rows 0-63
C_IDENT = C_WOIS + V     # [476, 504): I28 rows 0-27
C_E = C_IDENT + V        # [504, 508): E rows 0-27
C_TOT = C_E + W

# chunk -> summing engine ('v' DVE tensor_reduce, 's' ScalarE ACT accum_out).
# Chunks land in order every ~2.9us; alternating keeps both queues drained and
# puts the last-landing chunk on ScalarE (1.7us < DVE's 2.3us).
SUM_ENG = ['v', 's', 'v', 's', 'v', 's', 'v', 's']


def build_body(nc, xall, consts, yg, tc=None):
    t = tc
    from contextlib import ExitStack
    ctx = ExitStack()
    sbp = ctx.enter_context(t.tile_pool(name="sb", bufs=1))
    pp = ctx.enter_context(t.tile_pool(name="pp", bufs=1, space="PSUM"))

    MDT = F32R if USE_F32R else F32

    def st(shape, name, dt=F32):
        return sbp.tile(shape, dt, name=name, tag=name)

    xall_sb = st([128, DCH, XCW], "xall_sb", MDT)
    consts_sb = st([128, C_TOT], "consts_sb", MDT)
    sred = st([128, DCH], "sred")
    cbias = st([V, 1], "cbias")
    m_sb = st([64, NW + 2], "m_sb", MDT)
    scr_s = st([128, XCW - 3], "scr_s")
    scr_g = st([128, XCW - 3], "scr_g")
    dummy = st([1, 1], "dummy")

    psU = pp.tile([V, NW], F32, name="psU", tag="psU")
    psC = pp.tile([V, 1], F32, name="psC", tag="psC")
    psZ = pp.tile([V, NW], F32, name="psZ", tag="psZ")

    def f32c(ap):
        return ap.bitcast(F32) if USE_F32R else ap

    uot = lambda c: consts_sb[:, C_UOT + c * V:C_UOT + (c + 1) * V]
    cot = lambda c: consts_sb[:, C_COT + c * V:C_COT + (c + 1) * V]
    wois = consts_sb[0:64, C_WOIS:C_WOIS + V]
    ident = consts_sb[0:V, C_IDENT:C_IDENT + V]
    esrc = consts_sb[0:V, C_E:C_E + W]

    # Early dummy sigmoid so the ACT table load happens off the critical path.
    nc.vector.memset(dummy[:, :], 0.0)
    nc.scalar.activation(out=dummy[:, :], in_=dummy[:, :], func=AF.Sigmoid)
    # Y region must start as zeros (cold-start warmup state).
    nc.vector.memset(f32c(m_sb[:, :]), 0.0)

    # ---------------- DMAs: packed consts, then the 8 x chunk streams ------
    nc.sync.dma_start(consts_sb[:, :], consts)
    xv = xall.rearrange("p (c j) -> p c j", c=DCH)
    for c in range(DCH):
        nc.sync.dma_start(xall_sb[:, c, :], xv[:, c, :])

    # ---------------- U = Uo @ window, + E on the warmup cols --------------
    for c in range(DCH):
        nc.tensor.matmul(
            psU[:, :],
            lhsT=uot(c),
            rhs=xall_sb[:, c, 0:NW],
            start=(c == 0),
            stop=False,
        )
    nc.tensor.matmul(
        psU[:, 0:W], lhsT=f32c(ident), rhs=f32c(esrc), start=False, stop=True,
    )

    # ---------------- global sum (3 engines) + c ---------------------------
    # Each chunk's sum covers window cols [3,260) (every t in [t0-1, t0+254]
    # exactly once; cols 0-2 are U-halo duplicates) plus the complement —
    # together every timestep exactly once.
    for c in range(DCH):
        src = f32c(xall_sb[:, c, 3:XCW])
        dst = sred[:, c:c + 1]
        if SUM_ENG[c] == 'v':
            nc.vector.tensor_reduce(out=dst, in_=src,
                                    axis=mybir.AxisListType.X, op=ALU.add)
        elif SUM_ENG[c] == 's':
            nc.scalar.activation(out=scr_s[:, :], in_=src, func=AF.Copy,
                                 accum_out=dst)
        else:
            nc.gpsimd.tensor_scalar(out=scr_g[:, :], in0=src, scalar1=0.0,
                                    scalar2=0.0, op0=ALU.add, op1=ALU.add,
                                    accum_out=dst)
    for c in range(DCH):
        nc.tensor.matmul(
            psC[:, :],
            lhsT=f32c(cot(c)),
            rhs=sred[:, c:c + 1],
            start=(c == 0),
            stop=(c == DCH - 1),
        )
    nc.vector.tensor_copy(cbias[:, :], psC[:, :])

    # ---------------- assemble B' rows and sigmoid warm-init ---------------
    nc.scalar.activation(out=m_sb[32:32 + V, 0:NW], in_=psU[:, :],
                         func=AF.Identity, bias=cbias[:, 0:1], scale=1.0)
    nc.scalar.activation(out=m_sb[0:V, 1:NW + 1], in_=psU[:, :],
                         func=AF.Sigmoid, bias=cbias[:, 0:1], scale=1.0)

    # ---------------- Jacobi sweeps ---------------------------------------
    for _ in range(S_SWEEPS):
        nc.tensor.matmul(
            psZ[:, :],
            lhsT=wois,
            rhs=m_sb[0:64, 0:NW],
            start=True,
            stop=True,
        )
        nc.scalar.activation(out=m_sb[0:V, 1:NW + 1], in_=psZ[:, :],
                             func=AF.Sigmoid)

    # ---------------- write output ----------------------------------------
    nc.sync.dma_start(yg, f32c(m_sb[0:V, W:NW]))
    ctx.close()


_CACHED_NC = {}


def _get_nc():
    if "nc" not in _CACHED_NC:
        nc = bacc.Bacc("TRN2", target_bir_lowering=False, debug=False,
                       num_devices=N_CORES)
        MDT = F32R if USE_F32R else F32
        xall = nc.dram_tensor("xall", [128, DCH * XCW], MDT,
                              kind="ExternalInput")
        consts = nc.dram_tensor("consts", [128, C_TOT], MDT,
                                kind="ExternalInput")
        yg = nc.dram_tensor("yg", [V, TC], F32, kind="ExternalOutput")
        with tile.TileContext(nc) as t:
            build_body(nc, xall.ap(), consts.ap(), yg.ap(), tc=t)
        nc.compile()
        _CACHED_NC["nc"] = nc
    return _CACHED_NC["nc"]


def _to_dev_layout(buf):
    """(cols, D) -> (128, DCH*cols): dev[p, c*cols+j] = buf[j, 128c+p]."""
    cols = buf.shape[0]
    return np.ascontiguousarray(
        buf.T.reshape(DCH, 128, cols).transpose(1, 0, 2).reshape(128, -1))


def make_in_maps(x, Uo, Co, Wo):
    xb = np.ascontiguousarray(np.asarray(x, np.float32)[0])        # (T, D)
    Uo = np.asarray(Uo, np.float32)
    Co = np.asarray(Co, np.float32)
    Wo = np.asarray(Wo, np.float32)

    cbase = np.zeros((128, C_TOT), np.float32)
    cbase[:, C_UOT:C_UOT + DCH * V] = _to_dev_layout(Uo)
    cbase[:, C_COT:C_COT + DCH * V] = _to_dev_layout(Co)
    cbase[0:V, C_WOIS:C_WOIS + V] = Wo.T
    cbase[32:32 + V, C_WOIS:C_WOIS + V] = np.eye(V, dtype=np.float32)
    cbase[0:V, C_IDENT:C_IDENT + V] = np.eye(V, dtype=np.float32)

    in_maps = []
    for r in range(N_CORES):
        t0 = r * TC
        buf = np.zeros((XCW, D), np.float32)
        # window cols w=0..258 <-> x[t0-4+w]; col 259 stays zero
        lo = t0 - W
        src_lo = max(0, lo)
        buf[src_lo - lo:NW - 1] = xb[src_lo:t0 + TC - 1]
        # complement: every t outside [t0-1, t0+254]
        comp = np.concatenate([np.arange(0, max(0, t0 - 1)),
                               np.arange(t0 + TC - 1, T)])
        buf[NW:NW + len(comp)] = xb[comp]
        consts = cbase.copy()
        if r == 0:
            consts[0:V, C_E:C_E + W - 1] = E_NEG
        in_maps.append({"xall": _to_dev_layout(buf), "consts": consts})
    return in_maps


def unshard_output(results):
    y = np.empty((T, V), np.float32)
    for r in range(N_CORES):
        y[r * TC:(r + 1) * TC, :] = results[r]["yg"].T
    return y[None]


def run(inputs, trace=False, **kw):
    nc = _get_nc()
    in_maps = make_in_maps(inputs["x"], inputs["Uo"], inputs["Co"],
                           inputs["Wo"])
    res = bass_utils.run_bass_kernel_spmd(
        nc, in_maps, core_ids=list(range(N_CORES)), trace=trace, **kw)
    return unshard_output(res.results), res


def kernel(**inputs):
    out, _ = run(inputs)
    return out


# revision 10
# speedup vs baseline: 2.1679x; 1.0042x over previous
"""Trainium2 Bass kernel for nn_CascadedAttention_76836964925817.

Math: the reference module's attention machinery is dead code — softmax over a
size-1 axis is identically 1, so `context = x[0].sum(axis=0)` is a constant
and the layer reduces to the 28-dim nonlinear recurrence

    y[t] = sigmoid(Wo @ y[t-1] + Uo @ x[t-1] + c),   c = Co @ sum_t x[t],
    y[-1] = 0, x[-1] := 0.

The map y -> sigmoid(Wo y + b) is a strong contraction (measured Jacobian
2-norm <= 0.055), so each core solves its own 256-timestep slice from a cold
start with a W=4 column warmup — no cross-core state is needed.

Collective-free design (a collective's rendezvous wait absorbs inter-core
launch skew into the first core's measured exec time): every core receives
the FULL x (8.4 MB fp32, column-permuted on the host so one SPMD program
works for all cores) and computes the global sum itself:

  * per-core input xall (128, 8, 2054): d-major chunks; cols [0,260) are the
    core's local window x[t0-4 .. t0+254] (fed to the U matmuls), cols
    [260,2054) are all remaining timesteps in arbitrary order.
  * The global sum_t x[t] is one free-axis sum per chunk over cols [3,2054),
    split across VectorE (tensor_reduce), ScalarE (activation accum_out) and
    GpSimd (tensor_scalar accum_out) so every chunk's sum hides behind the
    per-chunk DMA stream; the last-arriving chunk goes to a fast engine.
  * U window: 8 f32r matmuls (Uo.T chunks vs window cols) accumulated in
    PSUM; one extra identity matmul adds E (E = -500 on warmup cols for
    core 0 only, making its pre-t=0 state decay to the true zero init).
  * c: 8 tiny f32 matmuls Co.T chunks vs the per-chunk sums.
  * recurrence: f32r keeps only ~13 mantissa bits, so the big constant c
    (|c| <= 140) never enters the f32r moving stack — it rides the sweep
    ACT's per-partition f32 bias instead.  Rows 32-59 carry only U (+E)
    with |U| <= 7.1 (f32r rounding ~8e-4, within budget).  A sigmoid
    warm-init ACT seeds Y (rows 0-27); then S=2 Jacobi sweeps, each ONE f32r
    matmul with the constant stationary [Wo.T;0;I;0] over the stacked [Y; U]
    window plus one sigmoid ACT with bias=c.  Output is Y cols [4,260).

All constants ride in a single packed (128, 508) tensor -> one DMA.
"""

import numpy as np

import concourse.bass as bass
import concourse.mybir as mybir
import concourse.tile as tile
from concourse import bacc
from concourse import bass_utils

F32 = mybir.dt.float32
F32R = mybir.dt.float32r
AF = mybir.ActivationFunctionType
ALU = mybir.AluOpType

T, D, V = 2048, 1024, 28
N_CORES = 8
TC = T // N_CORES        # 256 output timesteps per core
W = 4                    # warmup columns
NW = TC + W              # 260 window columns (U matmul width, even)
XCW = 2054               # per-chunk input cols: 260 window + 1794 complement
DCH = D // 128           # 8 contraction chunks
S_SWEEPS = 2             # Jacobi sweeps after the sigmoid warm-init
E_NEG = -500.0           # warmup bias (must be < -(max|c| + margin) ~ -170)
USE_F32R = True

# packed consts layout (128, 508)
C_UOT = 0                # [0, 224): Uo.T chunks
C_COT = DCH * V          # [224, 448): Co.T chunks
C_WOIS = 2 * DCH * V     # [448, 476): [Wo.T;0; I;0] rows 0-63
C_IDENT = C_WOIS + V     # [476, 504): I28 rows 0-27
C_E = C_IDENT + V        # [504, 508): E rows 0-27
C_TOT = C_E + W

# chunk -> summing engine ('v' DVE tensor_reduce, 's' ScalarE ACT accum_out).
# Chunks land in order every ~2.9us; alternating keeps both queues drained and
# puts the last-landing chunk on ScalarE (1.7us < DVE's 2.3us).
SUM_ENG = ['v', 's', 'v', 's', 'v', 's', 'v', 's']


def build_body(nc, xall, consts, yg, tc=None):
    t = tc
    from contextlib import ExitStack
    ctx = ExitStack()
    sbp = ctx.enter_context(t.tile_pool(name="sb", bufs=1))
    pp = ctx.enter_context(t.tile_pool(name="pp", bufs=1, space="PSUM"))

    MDT = F32R if USE_F32R else F32

    def st(shape, name, dt=F32):
        return sbp.tile(shape, dt, name=name, tag=name)

    xall_sb = st([128, DCH, XCW], "xall_sb", MDT)
    consts_sb = st([128, C_TOT], "consts_sb", MDT)
    sred = st([128, DCH], "sred")
    cbias = st([V, 1], "cbias")
    m_sb = st([64, NW + 2], "m_sb", MDT)
    scr_s = st([128, XCW - 3], "scr_s")
    scr_g = st([128, XCW - 3], "scr_g")
    dummy = st([1, 1], "dummy")

    psU = pp.tile([V, NW], F32, name="psU", tag="psU")
    psC = pp.tile([V, 1], F32, name="psC", tag="psC")
    psZ = pp.tile([V, NW], F32, name="psZ", tag="psZ")

    def f32c(ap):
        return ap.bitcast(F32) if USE_F32R else ap

    uot = lambda c: consts_sb[:, C_UOT + c * V:C_UOT + (c + 1) * V]
    cot = lambda c: consts_sb[:, C_COT + c * V:C_COT + (c + 1) * V]
    wois = consts_sb[0:64, C_WOIS:C_WOIS + V]
    ident = consts_sb[0:V, C_IDENT:C_IDENT + V]
    esrc = consts_sb[0:V, C_E:C_E + W]

    # Early dummy sigmoid so the ACT table load happens off the critical path.
    nc.vector.memset(dummy[:, :], 0.0)
    nc.scalar.activation(out=dummy[:, :], in_=dummy[:, :], func=AF.Sigmoid)
    # Y region must start as zeros (cold-start warmup state).
    nc.vector.memset(f32c(m_sb[:, :]), 0.0)

    # ---------------- DMAs: packed consts, then the 8 x chunk streams ------
    nc.sync.dma_start(consts_sb[:, :], consts)
    xv = xall.rearrange("p (c j) -> p c j", c=DCH)
    for c in range(DCH):
        nc.sync.dma_start(xall_sb[:, c, :], xv[:, c, :])

    # ---------------- U = Uo @ window, + E on the warmup cols --------------
    for c in range(DCH):
        nc.tensor.matmul(
            psU[:, :],
            lhsT=f32c(uot(c)),
            rhs=f32c(xall_sb[:, c, 0:NW]),
            start=(c == 0),
            stop=False,
        )
    nc.tensor.matmul(
        psU[:, 0:W], lhsT=f32c(ident), rhs=f32c(esrc), start=False, stop=True,
    )

    # ---------------- global sum (3 engines) + c ---------------------------
    # Each chunk's sum covers window cols [3,260) (every t in [t0-1, t0+254]
    # exactly once; cols 0-2 are U-halo duplicates) plus the complement —
    # together every timestep exactly once.
    for c in range(DCH):
        src = f32c(xall_sb[:, c, 3:XCW])
        dst = sred[:, c:c + 1]
        if SUM_ENG[c] == 'v':
            nc.vector.tensor_reduce(out=dst, in_=src,
                                    axis=mybir.AxisListType.X, op=ALU.add)
        elif SUM_ENG[c] == 's':
            nc.scalar.activation(out=scr_s[:, :], in_=src, func=AF.Copy,
                                 accum_out=dst)
        else:
            nc.gpsimd.tensor_scalar(out=scr_g[:, :], in0=src, scalar1=0.0,
                                    scalar2=0.0, op0=ALU.add, op1=ALU.add,
                                    accum_out=dst)
    for c in range(DCH):
        nc.tensor.matmul(
            psC[:, :],
            lhsT=f32c(cot(c)),
            rhs=sred[:, c:c + 1],
            start=(c == 0),
            stop=(c == DCH - 1),
        )
    nc.vector.tensor_copy(cbias[:, :], psC[:, :])

    # ------------- U row (c stays out of the f32r stack) + warm-init -------
    nc.scalar.activation(out=m_sb[32:32 + V, 0:NW], in_=psU[:, :],
                         func=AF.Copy)
    nc.scalar.activation(out=m_sb[0:V, 1:NW + 1], in_=psU[:, :],
                         func=AF.Sigmoid, bias=cbias[:, 0:1], scale=1.0)

    # ---------------- Jacobi sweeps ---------------------------------------
    for _ in range(S_SWEEPS):
        nc.tensor.matmul(
            psZ[:, :],
            lhsT=wois,
            rhs=m_sb[0:64, 0:NW],
            start=True,
            stop=True,
        )
        nc.scalar.activation(out=m_sb[0:V, 1:NW + 1], in_=psZ[:, :],
                             func=AF.Sigmoid, bias=cbias[:, 0:1], scale=1.0)

    # ---------------- write output ----------------------------------------
    nc.sync.dma_start(yg, f32c(m_sb[0:V, W:NW]))
    ctx.close()


_CACHED_NC = {}


def _get_nc():
    if "nc" not in _CACHED_NC:
        nc = bacc.Bacc("TRN2", target_bir_lowering=False, debug=False,
                       num_devices=N_CORES)
        MDT = F32R if USE_F32R else F32
        xall = nc.dram_tensor("xall", [128, DCH * XCW], MDT,
                              kind="ExternalInput")
        consts = nc.dram_tensor("consts", [128, C_TOT], MDT,
                                kind="ExternalInput")
        yg = nc.dram_tensor("yg", [V, TC], F32, kind="ExternalOutput")
        with tile.TileContext(nc) as t:
            build_body(nc, xall.ap(), consts.ap(), yg.ap(), tc=t)
        nc.compile()
        _CACHED_NC["nc"] = nc
    return _CACHED_NC["nc"]


def _to_dev_layout(buf):
    """(cols, D) -> (128, DCH*cols): dev[p, c*cols+j] = buf[j, 128c+p]."""
    cols = buf.shape[0]
    return np.ascontiguousarray(
        buf.T.reshape(DCH, 128, cols).transpose(1, 0, 2).reshape(128, -1))


def make_in_maps(x, Uo, Co, Wo):
    xb = np.ascontiguousarray(np.asarray(x, np.float32)[0])        # (T, D)
    Uo = np.asarray(Uo, np.float32)
    Co = np.asarray(Co, np.float32)
    Wo = np.asarray(Wo, np.float32)

    cbase = np.zeros((128, C_TOT), np.float32)
    cbase[:, C_UOT:C_UOT + DCH * V] = _to_dev_layout(Uo)
    cbase[:, C_COT:C_COT + DCH * V] = _to_dev_layout(Co)
    cbase[0:V, C_WOIS:C_WOIS + V] = Wo.T
    cbase[32:32 + V, C_WOIS:C_WOIS + V] = np.eye(V, dtype=np.float32)
    cbase[0:V, C_IDENT:C_IDENT + V] = np.eye(V, dtype=np.float32)

    in_maps = []
    for r in range(N_CORES):
        t0 = r * TC
        buf = np.zeros((XCW, D), np.float32)
        # window cols w=0..258 <-> x[t0-4+w]; col 259 stays zero
        lo = t0 - W
        src_lo = max(0, lo)
        buf[src_lo - lo:NW - 1] = xb[src_lo:t0 + TC - 1]
        # complement: every t outside [t0-1, t0+254]
        comp = np.concatenate([np.arange(0, max(0, t0 - 1)),
                               np.arange(t0 + TC - 1, T)])
        buf[NW:NW + len(comp)] = xb[comp]
        consts = cbase.copy()
        if r == 0:
            consts[0:V, C_E:C_E + W - 1] = E_NEG
        in_maps.append({"xall": _to_dev_layout(buf), "consts": consts})
    return in_maps


def unshard_output(results):
    y = np.empty((T, V), np.float32)
    for r in range(N_CORES):
        y[r * TC:(r + 1) * TC, :] = results[r]["yg"].T
    return y[None]


def run(inputs, trace=False, **kw):
    nc = _get_nc()
    in_maps = make_in_maps(inputs["x"], inputs["Uo"], inputs["Co"],
                           inputs["Wo"])
    res = bass_utils.run_bass_kernel_spmd(
        nc, in_maps, core_ids=list(range(N_CORES)), trace=trace, **kw)
    return unshard_output(res.results), res


def kernel(**inputs):
    out, _ = run(inputs)
    return out


# revision 11
# speedup vs baseline: 2.2318x; 1.0295x over previous
"""Trainium2 Bass kernel for nn_CascadedAttention_76836964925817.

Math: the reference module's attention machinery is dead code — softmax over a
size-1 axis is identically 1, so `context = x[0].sum(axis=0)` is a constant
and the layer reduces to the 28-dim nonlinear recurrence

    y[t] = sigmoid(Wo @ y[t-1] + Uo @ x[t-1] + c),   c = Co @ sum_t x[t],
    y[-1] = 0, x[-1] := 0.

The map y -> sigmoid(Wo y + b) is a strong contraction (measured Jacobian
2-norm <= 0.055), so each core solves its own 256-timestep slice from a cold
start with a W=4 column warmup — no cross-core state is needed.

Collective-free design (a collective's rendezvous wait absorbs inter-core
launch skew into the first core's measured exec time): every core receives
the FULL x (8.4 MB fp32, column-permuted on the host so one SPMD program
works for all cores) and computes the global sum itself:

  * per-core input xall (128, 8, 2054): d-major chunks; cols [0,260) are the
    core's local window x[t0-4 .. t0+254] (fed to the U matmuls), cols
    [260,2054) are all remaining timesteps in arbitrary order.
  * The global sum_t x[t] is one free-axis sum per chunk over cols [3,2054),
    split across VectorE (tensor_reduce), ScalarE (activation accum_out) and
    GpSimd (tensor_scalar accum_out) so every chunk's sum hides behind the
    per-chunk DMA stream; the last-arriving chunk goes to a fast engine.
  * U window: 8 f32r matmuls (Uo.T chunks vs window cols) accumulated in
    PSUM; one extra identity matmul adds E (E = -500 on warmup cols for
    core 0 only, making its pre-t=0 state decay to the true zero init).
  * c: 8 tiny f32 matmuls Co.T chunks vs the per-chunk sums.
  * recurrence: f32r keeps only ~13 mantissa bits, so the big constant c
    (|c| <= 140) never enters the f32r moving stack — it rides the sweep
    ACT's per-partition f32 bias instead.  Rows 32-59 carry only U (+E)
    with |U| <= 7.1 (f32r rounding ~8e-4, within budget).  A sigmoid
    warm-init ACT seeds Y (rows 0-27); then S=2 Jacobi sweeps, each ONE f32r
    matmul with the constant stationary [Wo.T;0;I;0] over the stacked [Y; U]
    window plus one sigmoid ACT with bias=c.  Output is Y cols [4,260).

All constants ride in a single packed (128, 508) tensor -> one DMA.
"""

import numpy as np

import concourse.bass as bass
import concourse.mybir as mybir
import concourse.tile as tile
from concourse import bacc
from concourse import bass_utils

F32 = mybir.dt.float32
F32R = mybir.dt.float32r
AF = mybir.ActivationFunctionType
ALU = mybir.AluOpType

T, D, V = 2048, 1024, 28
N_CORES = 8
TC = T // N_CORES        # 256 output timesteps per core
W = 4                    # warmup columns
NW = TC + W              # 260 window columns (U matmul width, even)
XCW = 2054               # per-chunk input cols: 260 window + 1794 complement
DCH = D // 128           # 8 contraction chunks
S_SWEEPS = 2             # Jacobi sweeps after the sigmoid warm-init
E_NEG = -500.0           # warmup bias (must be < -(max|c| + margin) ~ -170)
USE_F32R = False

# packed consts layout (128, 508)
C_UOT = 0                # [0, 224): Uo.T chunks
C_COT = DCH * V          # [224, 448): Co.T chunks
C_WOIS = 2 * DCH * V     # [448, 476): [Wo.T;0; I;0] rows 0-63
C_IDENT = C_WOIS + V     # [476, 504): I28 rows 0-27
C_E = C_IDENT + V        # [504, 508): E rows 0-27
C_TOT = C_E + W

# chunk -> summing engine ('v' DVE tensor_reduce, 's' ScalarE ACT accum_out).
# Chunks land in order every ~2.9us; alternating keeps both queues drained and
# puts the last-landing chunk on ScalarE (1.7us < DVE's 2.3us).
SUM_ENG = ['v', 's', 'v', 's', 'v', 's', 'v', 's']


def build_body(nc, xall, consts, yg, tc=None):
    t = tc
    from contextlib import ExitStack
    ctx = ExitStack()
    sbp = ctx.enter_context(t.tile_pool(name="sb", bufs=1))
    pp = ctx.enter_context(t.tile_pool(name="pp", bufs=1, space="PSUM"))

    MDT = F32R if USE_F32R else F32

    def st(shape, name, dt=F32):
        return sbp.tile(shape, dt, name=name, tag=name)

    xall_sb = st([128, DCH, XCW], "xall_sb", MDT)
    consts_sb = st([128, C_TOT], "consts_sb", MDT)
    sred = st([128, DCH], "sred")
    cbias = st([V, 1], "cbias")
    m_sb = st([64, NW + 2], "m_sb", MDT)
    scr_s = st([128, XCW - 3], "scr_s")
    scr_g = st([128, XCW - 3], "scr_g")
    dummy = st([1, 1], "dummy")

    psU = pp.tile([V, NW], F32, name="psU", tag="psU")
    psC = pp.tile([V, 1], F32, name="psC", tag="psC")
    psZ = pp.tile([V, NW], F32, name="psZ", tag="psZ")

    def f32c(ap):
        return ap.bitcast(F32) if USE_F32R else ap

    uot = lambda c: consts_sb[:, C_UOT + c * V:C_UOT + (c + 1) * V]
    cot = lambda c: consts_sb[:, C_COT + c * V:C_COT + (c + 1) * V]
    wois = consts_sb[0:64, C_WOIS:C_WOIS + V]
    ident = consts_sb[0:V, C_IDENT:C_IDENT + V]
    esrc = consts_sb[0:V, C_E:C_E + W]

    # Early dummy sigmoid so the ACT table load happens off the critical path.
    nc.vector.memset(dummy[:, :], 0.0)
    nc.scalar.activation(out=dummy[:, :], in_=dummy[:, :], func=AF.Sigmoid)
    # Y region must start as zeros (cold-start warmup state).
    nc.vector.memset(f32c(m_sb[:, :]), 0.0)

    # ---------------- DMAs: packed consts, then the 8 x chunk streams ------
    nc.sync.dma_start(consts_sb[:, :], consts)
    xv = xall.rearrange("p (c j) -> p c j", c=DCH)
    for c in range(DCH):
        nc.sync.dma_start(xall_sb[:, c, :], xv[:, c, :])

    # ---------------- U = Uo @ window, + E on the warmup cols --------------
    for c in range(DCH):
        nc.tensor.matmul(
            psU[:, :],
            lhsT=f32c(uot(c)),
            rhs=f32c(xall_sb[:, c, 0:NW]),
            start=(c == 0),
            stop=False,
        )
    nc.tensor.matmul(
        psU[:, 0:W], lhsT=f32c(ident), rhs=f32c(esrc), start=False, stop=True,
    )

    # ---------------- global sum (3 engines) + c ---------------------------
    # Each chunk's sum covers window cols [3,260) (every t in [t0-1, t0+254]
    # exactly once; cols 0-2 are U-halo duplicates) plus the complement —
    # together every timestep exactly once.
    for c in range(DCH):
        src = f32c(xall_sb[:, c, 3:XCW])
        dst = sred[:, c:c + 1]
        if SUM_ENG[c] == 'v':
            nc.vector.tensor_reduce(out=dst, in_=src,
                                    axis=mybir.AxisListType.X, op=ALU.add)
        elif SUM_ENG[c] == 's':
            nc.scalar.activation(out=scr_s[:, :], in_=src, func=AF.Copy,
                                 accum_out=dst)
        else:
            nc.gpsimd.tensor_scalar(out=scr_g[:, :], in0=src, scalar1=0.0,
                                    scalar2=0.0, op0=ALU.add, op1=ALU.add,
                                    accum_out=dst)
    for c in range(DCH):
        nc.tensor.matmul(
            psC[:, :],
            lhsT=f32c(cot(c)),
            rhs=sred[:, c:c + 1],
            start=(c == 0),
            stop=(c == DCH - 1),
        )
    nc.vector.tensor_copy(cbias[:, :], psC[:, :])

    # ------------- U row (c stays out of the f32r stack) + warm-init -------
    nc.scalar.activation(out=m_sb[32:32 + V, 0:NW], in_=psU[:, :],
                         func=AF.Copy)
    nc.scalar.activation(out=m_sb[0:V, 1:NW + 1], in_=psU[:, :],
                         func=AF.Sigmoid, bias=cbias[:, 0:1], scale=1.0)

    # ---------------- Jacobi sweeps ---------------------------------------
    for _ in range(S_SWEEPS):
        nc.tensor.matmul(
            psZ[:, :],
            lhsT=wois,
            rhs=m_sb[0:64, 0:NW],
            start=True,
            stop=True,
        )
        nc.scalar.activation(out=m_sb[0:V, 1:NW + 1], in_=psZ[:, :],
                             func=AF.Sigmoid, bias=cbias[:, 0:1], scale=1.0)

    # ---------------- write output ----------------------------------------
    nc.sync.dma_start(yg, f32c(m_sb[0:V, W:NW]))
    ctx.close()


_CACHED_NC = {}


def _get_nc():
    if "nc" not in _CACHED_NC:
        nc = bacc.Bacc("TRN2", target_bir_lowering=False, debug=False,
                       num_devices=N_CORES)
        MDT = F32R if USE_F32R else F32
        xall = nc.dram_tensor("xall", [128, DCH * XCW], MDT,
                              kind="ExternalInput")
        consts = nc.dram_tensor("consts", [128, C_TOT], MDT,
                                kind="ExternalInput")
        yg = nc.dram_tensor("yg", [V, TC], F32, kind="ExternalOutput")
        with tile.TileContext(nc) as t:
            build_body(nc, xall.ap(), consts.ap(), yg.ap(), tc=t)
        nc.compile()
        _CACHED_NC["nc"] = nc
    return _CACHED_NC["nc"]


def _to_dev_layout(buf):
    """(cols, D) -> (128, DCH*cols): dev[p, c*cols+j] = buf[j, 128c+p]."""
    cols = buf.shape[0]
    return np.ascontiguousarray(
        buf.T.reshape(DCH, 128, cols).transpose(1, 0, 2).reshape(128, -1))


def make_in_maps(x, Uo, Co, Wo):
    xb = np.ascontiguousarray(np.asarray(x, np.float32)[0])        # (T, D)
    Uo = np.asarray(Uo, np.float32)
    Co = np.asarray(Co, np.float32)
    Wo = np.asarray(Wo, np.float32)

    cbase = np.zeros((128, C_TOT), np.float32)
    cbase[:, C_UOT:C_UOT + DCH * V] = _to_dev_layout(Uo)
    cbase[:, C_COT:C_COT + DCH * V] = _to_dev_layout(Co)
    cbase[0:V, C_WOIS:C_WOIS + V] = Wo.T
    cbase[32:32 + V, C_WOIS:C_WOIS + V] = np.eye(V, dtype=np.float32)
    cbase[0:V, C_IDENT:C_IDENT + V] = np.eye(V, dtype=np.float32)

    in_maps = []
    for r in range(N_CORES):
        t0 = r * TC
        buf = np.zeros((XCW, D), np.float32)
        # window cols w=0..258 <-> x[t0-4+w]; col 259 stays zero
        lo = t0 - W
        src_lo = max(0, lo)
        buf[src_lo - lo:NW - 1] = xb[src_lo:t0 + TC - 1]
        # complement: every t outside [t0-1, t0+254]
        comp = np.concatenate([np.arange(0, max(0, t0 - 1)),
                               np.arange(t0 + TC - 1, T)])
        buf[NW:NW + len(comp)] = xb[comp]
        consts = cbase.copy()
        if r == 0:
            consts[0:V, C_E:C_E + W - 1] = E_NEG
        in_maps.append({"xall": _to_dev_layout(buf), "consts": consts})
    return in_maps


def unshard_output(results):
    y = np.empty((T, V), np.float32)
    for r in range(N_CORES):
        y[r * TC:(r + 1) * TC, :] = results[r]["yg"].T
    return y[None]


def run(inputs, trace=False, **kw):
    nc = _get_nc()
    in_maps = make_in_maps(inputs["x"], inputs["Uo"], inputs["Co"],
                           inputs["Wo"])
    res = bass_utils.run_bass_kernel_spmd(
        nc, in_maps, core_ids=list(range(N_CORES)), trace=trace, **kw)
    return unshard_output(res.results), res


def kernel(**inputs):
    out, _ = run(inputs)
    return out


# revision 14
# speedup vs baseline: 2.3671x; 1.0606x over previous
"""Trainium2 Bass kernel for nn_CascadedAttention_76836964925817.

Math: the reference module's attention machinery is dead code — softmax over a
size-1 axis is identically 1, so `context = x[0].sum(axis=0)` is a constant
and the layer reduces to the 28-dim nonlinear recurrence

    y[t] = sigmoid(Wo @ y[t-1] + Uo @ x[t-1] + c),   c = Co @ sum_t x[t],
    y[-1] = 0, x[-1] := 0.

The map y -> sigmoid(Wo y + b) is a strong contraction (measured Jacobian
2-norm <= 0.055), so each core solves its own 256-timestep slice from a cold
start with a W=4 column warmup — no cross-core state is needed.

Collective-free design (a collective's rendezvous wait absorbs inter-core
launch skew into the first core's measured exec time): every core receives
the FULL x (8.4 MB fp32, column-permuted on the host so one SPMD program
works for all cores) and computes the global sum itself:

  * per-core input xall (128, 8, 2054): d-major chunks; cols [0,260) are the
    core's local window x[t0-4 .. t0+254] (fed to the U matmuls), cols
    [260,2054) are all remaining timesteps in arbitrary order.
  * The global sum_t x[t] is one free-axis sum per chunk over cols [3,2054),
    split across VectorE (tensor_reduce) and ScalarE (activation accum_out)
    so every chunk's sum hides behind the per-chunk DMA stream.  The last
    chunk is DMA'd and summed in two pieces so the final c-dependency is a
    small 514-column tail.
  * U window: 8 fp32 matmuls (Uo.T chunks vs window cols) accumulated in
    PSUM; one extra identity matmul adds E (E = -500 on warmup cols for
    core 0 only, making its pre-t=0 state decay to the true zero init).
  * c: tiny fp32 matmuls Co.T chunks vs the per-chunk sums.
  * recurrence: f32r matmuls run 4x faster than fp32 but round operands to
    ~bf16 mantissa, so the moving stack is [Y; U_hi; U_lo] (96 rows): U_hi
    is the f32r-rounded copy of U, U_lo the exact residual (DVE subtract),
    and the large constant c rides the sweep ACT's per-partition f32 bias.
    A sigmoid warm-init ACT seeds Y; then S=2 Jacobi sweeps, each ONE f32r
    matmul with the constant stationary [Wo.T;0;I;0;I;0] plus one sigmoid
    ACT.  The final sigmoid writes a full-f32 output tile (so shipped y is
    not f32r-rounded) covering Y cols [4,260) = y[t0 .. t0+255].

All constants ride in a single packed (128, 508) tensor -> one DMA.
"""

import numpy as np

import concourse.bass as bass
import concourse.mybir as mybir
import concourse.tile as tile
from concourse import bacc
from concourse import bass_utils

F32 = mybir.dt.float32
F32R = mybir.dt.float32r
AF = mybir.ActivationFunctionType
ALU = mybir.AluOpType

T, D, V = 2048, 1024, 28
N_CORES = 8
TC = T // N_CORES        # 256 output timesteps per core
W = 4                    # warmup columns
NW = TC + W              # 260 window columns (U matmul width, even)
XCW = 2054               # per-chunk input cols: 260 window + 1794 complement
XSPL = 1540              # last chunk's DMA/sum split point
DCH = D // 128           # 8 contraction chunks
S_SWEEPS = 2             # Jacobi sweeps after the sigmoid warm-init
E_NEG = -500.0           # warmup bias (must be < -(max|c| + margin) ~ -170)

# packed consts layout (128, 508)
C_UOT = 0                # [0, 224): Uo.T chunks
C_COT = DCH * V          # [224, 448): Co.T chunks
C_WOIS = 2 * DCH * V     # [448, 476): [Wo.T;0; I;0; I;0] rows 0-95
C_IDENT = C_WOIS + V     # [476, 504): I28 rows 0-27
C_E = C_IDENT + V        # [504, 508): E rows 0-27
C_TOT = C_E + W


def build_body(nc, xall, consts, yg, tc=None):
    t = tc
    from contextlib import ExitStack
    ctx = ExitStack()
    sbp = ctx.enter_context(t.tile_pool(name="sb", bufs=1))
    pp = ctx.enter_context(t.tile_pool(name="pp", bufs=1, space="PSUM"))

    def st(shape, name, dt=F32):
        return sbp.tile(shape, dt, name=name, tag=name)

    xall_sb = st([128, DCH, XCW], "xall_sb")
    consts_sb = st([128, C_TOT], "consts_sb", F32R)
    sred = st([128, DCH + 1], "sred")
    cbias = st([V, 1], "cbias")
    m_sb = st([96, NW + 2], "m_sb", F32R)
    yout = st([V, TC], "yout")
    scr_s = st([128, XCW - 3], "scr_s")
    dummy = st([1, 1], "dummy")

    psU = pp.tile([V, NW], F32, name="psU", tag="psU")
    psC = pp.tile([V, 2], F32, name="psC", tag="psC")
    psZ = pp.tile([V, NW], F32, name="psZ", tag="psZ")

    cf = lambda ap: ap.bitcast(F32)
    uot = lambda c: cf(consts_sb[:, C_UOT + c * V:C_UOT + (c + 1) * V])
    cot = lambda c: cf(consts_sb[:, C_COT + c * V:C_COT + (c + 1) * V])
    wois = consts_sb[0:96, C_WOIS:C_WOIS + V]
    ident = cf(consts_sb[0:V, C_IDENT:C_IDENT + V])
    esrc = cf(consts_sb[0:V, C_E:C_E + W])

    # Early dummy sigmoid so the ACT table load happens off the critical path.
    nc.vector.memset(dummy[:, :], 0.0)
    nc.scalar.activation(out=dummy[:, :], in_=dummy[:, :], func=AF.Sigmoid)
    # Y region must start as zeros (cold-start warmup state).
    nc.vector.memset(cf(m_sb[:, :]), 0.0)

    # ---------------- DMAs: packed consts, then the x chunk stream ---------
    nc.sync.dma_start(consts_sb[:, :], consts)
    xv = xall.rearrange("p (c j) -> p c j", c=DCH)
    for c in range(DCH - 1):
        nc.sync.dma_start(xall_sb[:, c, :], xv[:, c, :])
    nc.sync.dma_start(xall_sb[:, DCH - 1, 0:XSPL], xv[:, DCH - 1, 0:XSPL])
    nc.sync.dma_start(xall_sb[:, DCH - 1, XSPL:XCW], xv[:, DCH - 1, XSPL:XCW])

    # ---------------- U = Uo @ window, + E on the warmup cols --------------
    for c in range(DCH):
        nc.tensor.matmul(
            psU[:, :],
            lhsT=uot(c),
            rhs=xall_sb[:, c, 0:NW],
            start=(c == 0),
            stop=False,
        )
    nc.tensor.matmul(
        psU[:, 0:W], lhsT=ident, rhs=esrc, start=False, stop=True,
    )

    # ---------------- global sum (DVE + ScalarE) + c -----------------------
    # Each chunk's sum covers window cols [3,260) (every t in [t0-1, t0+254]
    # exactly once; cols 0-2 are U-halo duplicates) plus the complement —
    # together every timestep exactly once.  Chunks alternate engines; the
    # split last chunk puts only a 514-col tail on the critical path.
    def emit_sum(eng, src, dst):
        if eng == 'v':
            nc.vector.tensor_reduce(out=dst, in_=src,
                                    axis=mybir.AxisListType.X, op=ALU.add)
        else:
            nc.scalar.activation(out=scr_s[:, 0:src.shape[-1]], in_=src,
                                 func=AF.Copy, accum_out=dst)

    for c in range(DCH - 1):
        emit_sum('v' if c % 2 == 0 else 's',
                 xall_sb[:, c, 3:XCW], sred[:, c:c + 1])
    emit_sum('v', xall_sb[:, DCH - 1, 3:XSPL], sred[:, DCH - 1:DCH])
    emit_sum('s', xall_sb[:, DCH - 1, XSPL:XCW], sred[:, DCH:DCH + 1])

    for c in range(DCH):
        last = c == DCH - 1
        nc.tensor.matmul(
            psC[:, 0:2] if last else psC[:, 0:1],
            lhsT=cot(c),
            rhs=sred[:, c:c + 2] if last else sred[:, c:c + 1],
            start=(c == 0),
            stop=last,
        )
    nc.vector.tensor_reduce(out=cbias[:, :], in_=psC[:, 0:2],
                            axis=mybir.AxisListType.X, op=ALU.add)

    # -------- stage U into the f32r stack as hi + exact residual lo --------
    nc.vector.tensor_copy(m_sb[32:32 + V, 0:NW], psU[:, :])
    nc.vector.tensor_tensor(m_sb[64:64 + V, 0:NW], psU[:, :],
                            cf(m_sb[32:32 + V, 0:NW]), ALU.subtract)
    nc.scalar.activation(out=m_sb[0:V, 1:NW + 1], in_=psU[:, :],
                         func=AF.Sigmoid, bias=cbias[:, 0:1], scale=1.0)

    # ---------------- Jacobi sweeps ---------------------------------------
    for s in range(S_SWEEPS):
        nc.tensor.matmul(
            psZ[:, :],
            lhsT=wois,
            rhs=m_sb[0:96, 0:NW],
            start=True,
            stop=True,
        )
        if s < S_SWEEPS - 1:
            nc.scalar.activation(out=m_sb[0:V, 1:NW + 1], in_=psZ[:, :],
                                 func=AF.Sigmoid, bias=cbias[:, 0:1],
                                 scale=1.0)
        else:
            # final sweep writes full-f32 output (no f32r rounding of y)
            nc.scalar.activation(out=yout[:, :], in_=psZ[:, W - 1:NW - 1],
                                 func=AF.Sigmoid, bias=cbias[:, 0:1],
                                 scale=1.0)

    # ---------------- write output ----------------------------------------
    nc.sync.dma_start(yg, yout[:, :])
    ctx.close()


_CACHED_NC = {}


def _get_nc():
    if "nc" not in _CACHED_NC:
        nc = bacc.Bacc("TRN2", target_bir_lowering=False, debug=False,
                       num_devices=N_CORES)
        xall = nc.dram_tensor("xall", [128, DCH * XCW], F32,
                              kind="ExternalInput")
        consts = nc.dram_tensor("consts", [128, C_TOT], F32R,
                                kind="ExternalInput")
        yg = nc.dram_tensor("yg", [V, TC], F32, kind="ExternalOutput")
        with tile.TileContext(nc) as t:
            build_body(nc, xall.ap(), consts.ap(), yg.ap(), tc=t)
        nc.compile()
        _CACHED_NC["nc"] = nc
    return _CACHED_NC["nc"]


def _to_dev_layout(buf):
    """(cols, D) -> (128, DCH*cols): dev[p, c*cols+j] = buf[j, 128c+p]."""
    cols = buf.shape[0]
    return np.ascontiguousarray(
        buf.T.reshape(DCH, 128, cols).transpose(1, 0, 2).reshape(128, -1))


def make_in_maps(x, Uo, Co, Wo):
    xb = np.ascontiguousarray(np.asarray(x, np.float32)[0])        # (T, D)
    Uo = np.asarray(Uo, np.float32)
    Co = np.asarray(Co, np.float32)
    Wo = np.asarray(Wo, np.float32)

    cbase = np.zeros((128, C_TOT), np.float32)
    cbase[:, C_UOT:C_UOT + DCH * V] = _to_dev_layout(Uo)
    cbase[:, C_COT:C_COT + DCH * V] = _to_dev_layout(Co)
    cbase[0:V, C_WOIS:C_WOIS + V] = Wo.T
    cbase[32:32 + V, C_WOIS:C_WOIS + V] = np.eye(V, dtype=np.float32)
    cbase[64:64 + V, C_WOIS:C_WOIS + V] = np.eye(V, dtype=np.float32)
    cbase[0:V, C_IDENT:C_IDENT + V] = np.eye(V, dtype=np.float32)

    in_maps = []
    for r in range(N_CORES):
        t0 = r * TC
        buf = np.zeros((XCW, D), np.float32)
        # window cols w=0..258 <-> x[t0-4+w]; col 259 stays zero
        lo = t0 - W
        src_lo = max(0, lo)
        buf[src_lo - lo:NW - 1] = xb[src_lo:t0 + TC - 1]
        # complement: every t outside [t0-1, t0+254]
        comp = np.concatenate([np.arange(0, max(0, t0 - 1)),
                               np.arange(t0 + TC - 1, T)])
        buf[NW:NW + len(comp)] = xb[comp]
        consts = cbase.copy()
        if r == 0:
            consts[0:V, C_E:C_E + W - 1] = E_NEG
        in_maps.append({"xall": _to_dev_layout(buf), "consts": consts})
    return in_maps


def unshard_output(results):
    y = np.empty((T, V), np.float32)
    for r in range(N_CORES):
        y[r * TC:(r + 1) * TC, :] = results[r]["yg"].T
    return y[None]


def run(inputs, trace=False, **kw):
    nc = _get_nc()
    in_maps = make_in_maps(inputs["x"], inputs["Uo"], inputs["Co"],
                           inputs["Wo"])
    res = bass_utils.run_bass_kernel_spmd(
        nc, in_maps, core_ids=list(range(N_CORES)), trace=trace, **kw)
    return unshard_output(res.results), res


def kernel(**inputs):
    out, _ = run(inputs)
    return out
